# revision 1
# baseline (speedup 1.0000x reference)
"""Trainium2 Bass kernel for nn_Decoder (2-layer LSTM + 3 FC + top-k beam decode).

Strategy: pure data parallelism over batch (2048 -> 8 cores x 256).
All activations feature-major [feat, batch]. LSTM/fc1/fc2 in fp32 on PE;
fc3 (256->10000) as a 4-term bf16-split matmul (fp32 accuracy, smaller SBUF).
Argmax/top-k via DVE max8 + max_index; embedding gather via one-hot matmul.
Host assembles the [B,16,4,2] trajectory from per-step argmax indices.
"""
import numpy as np
import ml_dtypes

B, D, H = 2048, 256, 256
K4, QW, QL, DELTA = 4, 100, 100, 16
Q = QW * QL
NCORES = 8
BS = B // NCORES          # 256 rows per core
NT = 20                   # fc3 N-tiles of 500
TW = 500

_CACHE = {}


def _build_nc(delta=DELTA, dbg=False):
    import concourse.mybir as mybir
    import concourse.tile as tile
    import concourse.bacc as bacc
    from concourse.masks import make_identity

    F32 = mybir.dt.float32
    BF16 = mybir.dt.bfloat16
    U32 = mybir.dt.uint32
    I32 = mybir.dt.int32
    AF = mybir.ActivationFunctionType
    ALU = mybir.AluOpType

    nc = bacc.Bacc(None, target_bir_lowering=False, debug=False)

    def din(name, shape, dt=F32):
        return nc.dram_tensor(name, shape, dt, kind="ExternalInput")

    x_in = din("x_fm", [2, 128, BS])
    h1_in = din("h1_fm", [2, 128, BS])
    c1_in = din("c1_fm", [2, 128, BS])
    h2_in = din("h2_fm", [2, 128, BS])
    c2_in = din("c2_fm", [2, 128, BS])
    w1ih_in = din("w1ihT", [2, 128, 4 * H])
    w1hh_in = din("w1hhT", [2, 128, 4 * H])
    w2ih_in = din("w2ihT", [2, 128, 4 * H])
    w2hh_in = din("w2hhT", [2, 128, 4 * H])
    fc1_in = din("fc1T", [2, 128, H])
    fc2_in = din("fc2T", [2, 128, H])
    fc3h_in = din("fc3Th", [2, 128, Q], BF16)
    fc3l_in = din("fc3Tl", [2, 128, Q], BF16)
    fcqw_in = din("fcqwT", [100, 128])
    fcql_in = din("fcqlT", [100, 128])
    b1_in = din("b1r", [128, 8])
    b2_in = din("b2r", [128, 8])
    fc1b_in = din("fc1br", [128, 2])
    fc2b_in = din("fc2br", [128, 2])
    b3rep_in = din("b3rep", [128, Q])
    fcqwb_in = din("fcqwb", [128, 1])
    fcqlb_in = din("fcqlb", [128, 1])

    idx_out = nc.dram_tensor("idx_out", [2, 128, 20], U32, kind="ExternalOutput")
    if dbg:
        dbg_log = nc.dram_tensor("dbg_log", [128, Q], mybir.dt.float32,
                                 kind="ExternalOutput")
        dbg_h1 = nc.dram_tensor("dbg_h1", [128, 2, BS], mybir.dt.float32,
                                kind="ExternalOutput")
        dbg_y2 = nc.dram_tensor("dbg_y2", [128, 2, BS], mybir.dt.float32,
                                kind="ExternalOutput")
        dbg_x = nc.dram_tensor("dbg_x", [128, 2, BS], mybir.dt.float32,
                               kind="ExternalOutput")
        dbg_h1in = nc.dram_tensor("dbg_h1in", [128, 2, BS], mybir.dt.float32,
                                  kind="ExternalOutput")
        dbg_si = nc.dram_tensor("dbg_si", [128, 256], mybir.dt.float32,
                                kind="ExternalOutput")
        dbg_gp = nc.dram_tensor("dbg_gp", [128, 256], mybir.dt.float32,
                                kind="ExternalOutput")

    with tile.TileContext(nc) as tc:
        with (
            tc.tile_pool(name="wp", bufs=1) as wp,
            tc.tile_pool(name="st", bufs=1) as st,
            tc.tile_pool(name="wk", bufs=2) as wk,
            tc.tile_pool(name="ps", bufs=2, space="PSUM") as ps,
        ):
            # ---- load weights / consts ----
            def wload(src, shape, tag, dt=F32):
                t = wp.tile(shape, dt, tag=tag, name=tag)
                if len(shape) == 3 and shape[1] == 2:
                    nc.sync.dma_start(t[:], src[:].rearrange("c p f -> p c f"))
                else:
                    nc.sync.dma_start(t[:], src[:])
                return t

            w1ih = wload(w1ih_in, [128, 2, 4 * H], "w1ih")
            w1hh = wload(w1hh_in, [128, 2, 4 * H], "w1hh")
            w2ih = wload(w2ih_in, [128, 2, 4 * H], "w2ih")
            w2hh = wload(w2hh_in, [128, 2, 4 * H], "w2hh")
            fc1 = wload(fc1_in, [128, 2, H], "fc1")
            fc2 = wload(fc2_in, [128, 2, H], "fc2")
            fc3h = wload(fc3h_in, [128, 2, Q], "fc3h", BF16)
            fc3l = wload(fc3l_in, [128, 2, Q], "fc3l", BF16)
            fcqw = wload(fcqw_in, [100, 128], "fcqw")
            fcql = wload(fcql_in, [100, 128], "fcql")
            b1r = wload(b1_in, [128, 8], "b1r")
            b2r = wload(b2_in, [128, 8], "b2r")
            fc1b = wload(fc1b_in, [128, 2], "fc1b")
            fc2b = wload(fc2b_in, [128, 2], "fc2b")
            b3rep = wload(b3rep_in, [128, Q], "b3rep")
            fcqwb = wload(fcqwb_in, [128, 1], "fcqwb")
            fcqlb = wload(fcqlb_in, [128, 1], "fcqlb")

            ident = wp.tile([128, 128], F32)
            make_identity(nc, ident[:])
            io_f = wp.tile([128, 100], F32)
            nc.gpsimd.iota(io_f[:], pattern=[[1, 100]], base=0, channel_multiplier=0,
                           allow_small_or_imprecise_dtypes=True)
            io100 = wp.tile([128, 100], F32)
            nc.gpsimd.iota(io100[:], pattern=[[100, 100]], base=0,
                           channel_multiplier=0,
                           allow_small_or_imprecise_dtypes=True)
            io32 = wp.tile([128, 32], F32)
            nc.gpsimd.iota(io32[:], pattern=[[1, 32]], base=0, channel_multiplier=0,
                           allow_small_or_imprecise_dtypes=True)

            # ---- persistent states (feature-major [128, chunk, BS]) ----
            def sload(src, tag):
                t = st.tile([128, 2, BS], F32, tag=tag, name=tag)
                nc.sync.dma_start(t[:], src[:].rearrange("c p b -> p c b"))
                return t

            h1_t = sload(h1_in, "h1")
            c1_t = sload(c1_in, "c1")
            h2_t = sload(h2_in, "h2")
            c2_t = sload(c2_in, "c2")
            emb_t = st.tile([128, 2, BS], F32, tag="emb", name="emb")
            nc.sync.dma_start(emb_t[:], x_in[:].rearrange("c p b -> p c b"))
            outi = st.tile([128, 2, 20], U32, tag="outi", name="outi")
            nc.vector.memset(outi[:], 0)



            def pbig():
                return ps.tile([128, 4, 512], F32, tag="big", name="big")

            def lstm_layer(inp, hT, cT, wih, whh, br):
                gpt = pbig()

                def gsl(g):
                    return gpt[:, g // 2, (g % 2) * 256:(g % 2) * 256 + 256]

                for g in range(8):
                    sl = slice(128 * g, 128 * (g + 1))
                    nc.tensor.matmul(gsl(g), wih[:, 0, sl], inp[:, 0, :],
                                     start=True, stop=False)
                    nc.tensor.matmul(gsl(g), wih[:, 1, sl], inp[:, 1, :],
                                     start=False, stop=False)
                    nc.tensor.matmul(gsl(g), whh[:, 0, sl], hT[:, 0, :],
                                     start=False, stop=False)
                    nc.tensor.matmul(gsl(g), whh[:, 1, sl], hT[:, 1, :],
                                     start=False, stop=True)
                for ch in range(2):
                    si = wk.tile([128, 256], F32, tag="si", bufs=1)
                    sf = wk.tile([128, 256], F32, tag="sf", bufs=1)
                    tg = wk.tile([128, 256], F32, tag="tg", bufs=1)
                    so = wk.tile([128, 256], F32, tag="so", bufs=1)
                    if dbg and ch == 0 and wih is w1ih:
                        rawg = wk.tile([128, 256], F32, tag="rawg", name="rawg")
                        nc.scalar.copy(rawg[:], gsl(0))
                        nc.sync.dma_start(dbg_gp[:], rawg[:])
                    nc.scalar.activation(si[:], gsl(0 + ch), AF.Sigmoid,
                                         bias=br[:, 0 + ch:1 + ch])
                    if dbg and ch == 0 and wih is w1ih:
                        nc.sync.dma_start(dbg_si[:], si[:])
                    nc.scalar.activation(sf[:], gsl(2 + ch), AF.Sigmoid,
                                         bias=br[:, 2 + ch:3 + ch])
                    nc.scalar.activation(tg[:], gsl(4 + ch), AF.Tanh,
                                         bias=br[:, 4 + ch:5 + ch])
                    nc.scalar.activation(so[:], gsl(6 + ch), AF.Sigmoid,
                                         bias=br[:, 6 + ch:7 + ch])
                    t1 = wk.tile([128, 256], F32, tag="t1", bufs=1)
                    t2 = wk.tile([128, 256], F32, tag="t2", bufs=1)
                    nc.vector.tensor_mul(t1[:], sf[:], cT[:, ch, :])
                    nc.vector.tensor_mul(t2[:], si[:], tg[:])
                    nc.vector.tensor_add(cT[:, ch, :], t1[:], t2[:])
                    t3 = wk.tile([128, 256], F32, tag="t3", bufs=1)
                    nc.scalar.activation(t3[:], cT[:, ch, :], AF.Tanh)
                    nc.vector.tensor_mul(hT[:, ch, :], so[:], t3[:])

            if dbg:
                nc.sync.dma_start(dbg_x[:], emb_t[:])
                nc.sync.dma_start(dbg_h1in[:], h1_t[:])
            for t in range(delta):
                inp = emb_t
                lstm_layer(inp, h1_t, c1_t, w1ih, w1hh, b1r)
                lstm_layer(h1_t, h2_t, c2_t, w2ih, w2hh, b2r)

                if dbg and t == 0:
                    nc.sync.dma_start(dbg_h1[:], h1_t[:])
                # fc1, fc2 (feature-major out)
                y1 = st.tile([128, 2, BS], F32, tag="y1")
                y2 = st.tile([128, 2, BS], F32, tag="y2")
                for (dst, w, bb, src) in ((y1, fc1, fc1b, h2_t), (y2, fc2, fc2b, y1)):
                    fpt = pbig()
                    for m in range(2):
                        fsl = fpt[:, m // 2, (m % 2) * 256:(m % 2) * 256 + 256]
                        sl = slice(128 * m, 128 * (m + 1))
                        nc.tensor.matmul(fsl, w[:, 0, sl], src[:, 0, :],
                                         start=True, stop=False)
                        nc.tensor.matmul(fsl, w[:, 1, sl], src[:, 1, :],
                                         start=False, stop=True)
                        nc.scalar.activation(dst[:, m, :], fsl, AF.Identity,
                                             bias=bb[:, m:m + 1])

                if dbg and t == 0:
                    nc.sync.dma_start(dbg_y2[:], y2[:])
                # split y2 -> bf16 hi/lo
                y2h = st.tile([128, 2, BS], BF16, tag="y2h")
                y2l = st.tile([128, 2, BS], BF16, tag="y2l")
                nc.vector.tensor_copy(y2h[:], y2[:])
                nc.vector.tensor_sub(y2l[:], y2[:], y2h[:])

                # fc3 per batch-chunk: 5 groups x 4 tiles of 500
                ohwT = wk.tile([100, 256], F32, tag="ohwT", name="ohwT")
                ohlT = wk.tile([100, 256], F32, tag="ohlT", name="ohlT")
                for bc in range(2):
                    bsl = slice(128 * bc, 128 * (bc + 1))
                    lq = [wk.tile([128, 2500], F32, tag="logq", name="logq",
                                  bufs=2) for _ in range(4)]
                    cand_v = wk.tile([128, 32], F32, tag="candv", name="candv")
                    cand_i = wk.tile([128, 32], F32, tag="candi", name="candi")
                    nscan = [0]

                    def scan_ready(upto):
                        # scan any quarter fully evacuated below `upto`
                        while nscan[0] < 4 and (nscan[0] + 1) * 2500 <= upto:
                            qt = nscan[0]
                            m8q = wk.tile([128, 8], F32, tag="m8q", name="m8q")
                            i8q = wk.tile([128, 8], U32, tag="i8q", name="i8q")
                            nc.vector.max(m8q[:], lq[qt][:])
                            nc.vector.max_index(i8q[:], m8q[:], lq[qt][:])
                            nc.vector.tensor_copy(cand_v[:, 8 * qt:8 * qt + 8],
                                                  m8q[:])
                            i8f = wk.tile([128, 8], F32, tag="i8f", name="i8f")
                            nc.vector.tensor_copy(i8f[:], i8q[:])
                            nc.vector.tensor_scalar(
                                cand_i[:, 8 * qt:8 * qt + 8], i8f[:],
                                float(2500 * qt), None, op0=ALU.add)
                            nscan[0] += 1

                    for grp in range(5):
                        gp3 = pbig()
                        for tt in range(4):
                            n0 = (grp * 4 + tt) * TW
                            o = gp3[:, tt, 0:TW]
                            for k in range(2):
                                yhk = y2h[:, k, bsl]
                                ylk = y2l[:, k, bsl]
                                wh = fc3h[:, k, n0:n0 + TW]
                                wl = fc3l[:, k, n0:n0 + TW]
                                nc.tensor.matmul(o, yhk, wh, start=(k == 0),
                                                 stop=False)
                                nc.tensor.matmul(o, yhk, wl, start=False, stop=False)
                                nc.tensor.matmul(o, ylk, wh, start=False, stop=False)
                                nc.tensor.matmul(o, ylk, wl, start=False,
                                                 stop=(k == 1))
                        # evacuate per psum tile (+bias) into quarter tiles
                        for tt in range(4):
                            n0 = (grp * 4 + tt) * TW
                            qt = n0 // 2500
                            nc.vector.tensor_add(
                                lq[qt][:, n0 - 2500 * qt:n0 - 2500 * qt + TW],
                                gp3[:, tt, 0:TW],
                                b3rep[:, n0:n0 + TW])
                        if dbg and t == 0 and bc == 0:
                            for tt in range(4):
                                n0 = (grp * 4 + tt) * TW
                                qt = n0 // 2500
                                nc.sync.dma_start(
                                    dbg_log[:, n0:n0 + TW],
                                    lq[qt][:, n0 - 2500 * qt:n0 - 2500 * qt + TW])
                        scan_ready(grp * 2000 + 2000)

                    # merge 32 candidates
                    vm8 = wk.tile([128, 8], F32, tag="vm8", name="vm8")
                    pm8 = wk.tile([128, 8], U32, tag="pm8", name="pm8")
                    nc.vector.max(vm8[:], cand_v[:])
                    nc.vector.max_index(pm8[:], vm8[:], cand_v[:])
                    pmf = wk.tile([128, 8], F32, tag="pmf", name="pmf")
                    nc.vector.tensor_copy(pmf[:], pm8[:])
                    nk = 4 if t == 0 else 1
                    qsel = wk.tile([128, 4], F32, tag="qsel", name="qsel")
                    for kk in range(nk):
                        ohp = wk.tile([128, 32], F32, tag="ohp", name="ohp")
                        nc.vector.tensor_scalar(ohp[:], io32[:], pmf[:, kk:kk + 1],
                                                None, op0=ALU.is_equal)
                        tmq = wk.tile([128, 32], F32, tag="tmq", name="tmq")
                        nc.vector.tensor_mul(tmq[:], ohp[:], cand_i[:])
                        nc.vector.tensor_reduce(qsel[:, kk:kk + 1], tmq[:],
                                                axis=mybir.AxisListType.X,
                                                op=ALU.add)
                    if t == 0:
                        nc.vector.tensor_copy(outi[:, bc, 0:4], qsel[:, 0:4])
                    else:
                        nc.vector.tensor_copy(outi[:, bc, 4 + t - 1:5 + t - 1],
                                              qsel[:, 0:1])
                    if t == delta - 1:
                        continue
                    qf = wk.tile([128, 1], F32, tag="qf", name="qf")
                    nc.vector.tensor_copy(qf[:], qsel[:, 0:1])
                    # ohw[b,j] = (100j <= q) & (100j > q-100)
                    m_ge = wk.tile([128, 100], F32, tag="mge", name="mge", bufs=1)
                    nc.vector.tensor_scalar(m_ge[:], io100[:], qf[:], None,
                                            op0=ALU.is_le)
                    qm = wk.tile([128, 1], F32, tag="qm", name="qm")
                    nc.vector.tensor_scalar(qm[:], qf[:], -100.0, None, op0=ALU.add)
                    m_lt = wk.tile([128, 100], F32, tag="mlt", name="mlt", bufs=1)
                    nc.vector.tensor_scalar(m_lt[:], io100[:], qm[:], None,
                                            op0=ALU.is_gt)
                    ohw = wk.tile([128, 100], F32, tag="ohw", name="ohw", bufs=1)
                    nc.vector.tensor_mul(ohw[:], m_ge[:], m_lt[:])
                    tm = wk.tile([128, 100], F32, tag="tm", name="tm", bufs=1)
                    nc.vector.tensor_mul(tm[:], ohw[:], io_f[:])
                    fwf = wk.tile([128, 1], F32, tag="fwf", name="fwf")
                    nc.vector.tensor_reduce(fwf[:], tm[:], axis=mybir.AxisListType.X,
                                            op=ALU.add)
                    flf = wk.tile([128, 1], F32, tag="flf", name="flf")
                    nc.vector.tensor_scalar(flf[:], fwf[:], -100.0, qf[:],
                                            op0=ALU.mult, op1=ALU.add)
                    ohl = wk.tile([128, 100], F32, tag="ohl", name="ohl", bufs=1)
                    nc.vector.tensor_scalar(ohl[:], io_f[:], flf[:], None,
                                            op0=ALU.is_equal)
                    ptr = pbig()
                    pw = ptr[0:100, 0, 0:128]
                    nc.tensor.transpose(pw, ohw[:], ident[:])
                    nc.vector.tensor_copy(ohwT[:, bsl128(bc)], pw)
                    pl = ptr[0:100, 1, 0:128]
                    nc.tensor.transpose(pl, ohl[:], ident[:])
                    nc.vector.tensor_copy(ohlT[:, bsl128(bc)], pl)

                if t == delta - 1:
                    continue
                # embedding gather matmuls + bias
                pet = pbig()
                pe0 = pet[:, 0, 0:BS]
                pe1 = pet[:, 1, 0:BS]
                nc.tensor.matmul(pe0, fcqw[:], ohwT[:], start=True, stop=True)
                nc.tensor.matmul(pe1, fcql[:], ohlT[:], start=True, stop=True)
                nc.scalar.activation(emb_t[:, 0, :], pe0, AF.Identity,
                                     bias=fcqwb[:])
                nc.scalar.activation(emb_t[:, 1, :], pe1, AF.Identity,
                                     bias=fcqlb[:])

            for bc in range(2):
                nc.sync.dma_start(idx_out[bc], outi[:, bc, :])
    nc.finalize()
    return nc


def bsl128(bc):
    return slice(128 * bc, 128 * (bc + 1))


def _prep_shared(inputs):
    f32 = np.float32
    bf = ml_dtypes.bfloat16

    def fm(w):  # [out,in] -> lhsT layout [2,128,out]
        wt = np.ascontiguousarray(w.T.astype(f32))        # [in, out]
        return wt.reshape(2, 128, wt.shape[1])

    fc3T = np.ascontiguousarray(inputs["fc3_W"].T.astype(f32))  # [256, 10000]
    fc3h = fc3T.astype(bf)
    fc3l = (fc3T - fc3h.astype(f32)).astype(bf)

    shared = {
        "w1ihT": fm(inputs["lstm1_Wih"]),
        "w1hhT": fm(inputs["lstm1_Whh"]),
        "w2ihT": fm(inputs["lstm2_Wih"]),
        "w2hhT": fm(inputs["lstm2_Whh"]),
        "fc1T": fm(inputs["fc1_W"]),
        "fc2T": fm(inputs["fc2_W"]),
        "fc3Th": fc3h.reshape(2, 128, Q),
        "fc3Tl": fc3l.reshape(2, 128, Q),
        "fcqwT": np.ascontiguousarray(inputs["fcqw_W"].T.astype(f32))[:, :],
        "fcqlT": np.ascontiguousarray(inputs["fcql_W"].T.astype(f32))[:, :],
        "b1r": inputs["lstm1_b"].astype(f32).reshape(8, 128).T.copy(),
        "b2r": inputs["lstm2_b"].astype(f32).reshape(8, 128).T.copy(),
        "fc1br": inputs["fc1_b"].astype(f32).reshape(2, 128).T.copy(),
        "fc2br": inputs["fc2_b"].astype(f32).reshape(2, 128).T.copy(),
        "b3rep": np.ascontiguousarray(
            np.broadcast_to(inputs["fc3_b"].astype(f32), (128, Q))),
        "fcqwb": inputs["fcqw_b"].astype(f32).reshape(128, 1),
        "fcqlb": inputs["fcql_b"].astype(f32).reshape(128, 1),
    }
    return shared


def _per_core(inputs, c):
    f32 = np.float32
    sl = slice(c * BS, (c + 1) * BS)

    def fmT(a):  # [BS, 256] -> [2, 128, BS]
        return np.ascontiguousarray(a.T.astype(f32)).reshape(2, 128, BS)

    return {
        "x_fm": fmT(inputs["x"][sl, 0, :]),
        "h1_fm": fmT(inputs["h1"][0, sl]),
        "c1_fm": fmT(inputs["c1"][0, sl]),
        "h2_fm": fmT(inputs["h2"][0, sl]),
        "c2_fm": fmT(inputs["c2"][0, sl]),
    }


def kernel(**inputs):
    key = "nc"
    if key not in _CACHE:
        _CACHE[key] = _build_nc()
    nc = _CACHE[key]

    shared = _prep_shared(inputs)
    in_maps = []
    for c in range(NCORES):
        m = dict(shared)
        m.update(_per_core(inputs, c))
        in_maps.append(m)

    from concourse.bass_utils import run_bass_kernel_spmd
    res = run_bass_kernel_spmd(nc, in_maps, list(range(NCORES)))
    return assemble(res.results)


def assemble(results):
    traj = np.zeros((B, DELTA, K4, 2), np.float32)
    for c, r in enumerate(results):
        idx = r["idx_out"].reshape(2, 128, 20).astype(np.int64)
        for bc in range(2):
            rows = slice(c * BS + bc * 128, c * BS + (bc + 1) * 128)
            top4 = idx[bc, :, 0:4]
            traj[rows, 0, :, 0] = (top4 % QL).astype(np.float32)
            traj[rows, 0, :, 1] = (top4 // QL).astype(np.float32)
            greedy = idx[bc, :, 4:4 + DELTA - 1]
            traj[rows, 1:, 0, 0] = (greedy % QL).astype(np.float32)
            traj[rows, 1:, 0, 1] = (greedy // QL).astype(np.float32)
    return traj



# revision 19
# speedup vs baseline: 1.1792x; 1.1792x over previous
"""Trainium2 Bass kernel for nn_Decoder (2-layer LSTM + 3 FC + top-k decode).

Data parallel over batch (2048 -> 8 cores x 256). Activations feature-major
[feat, batch]. LSTM/fc1/fc2 fp32 on PE (exactness for the recurrence);
fc3 (256->10000) as 3-term bf16 split (err ~1e-6 rel, enough for argmax).
Argmax: fused DVE tensor_tensor_reduce (bias add + per-group max) evacuates
psum -> lq, winner group located from the 5 group maxes, gpsimd
indirect_copy gathers the winning 2000-wide window, short max_index gives
the index. Host assembles the [B,16,4,2] trajectory from indices.
"""
import numpy as np
import ml_dtypes

B, D, H = 2048, 256, 256
K4, QW, QL, DELTA = 4, 100, 100, 16
Q = QW * QL
NCORES = 8
BS = B // NCORES          # 256 rows per core
TW = 500                  # fc3 N-tile width
NG = 5                    # fc3 groups of 4 tiles (2000 cols)
GW = 4 * TW               # group width

_CACHE = {}


def _build_nc(delta=DELTA):
    import concourse.mybir as mybir
    import concourse.tile as tile
    import concourse.bacc as bacc
    from concourse.masks import make_identity

    F32 = mybir.dt.float32
    BF16 = mybir.dt.bfloat16
    U32 = mybir.dt.uint32
    U16 = mybir.dt.uint16
    AF = mybir.ActivationFunctionType
    ALU = mybir.AluOpType

    nc = bacc.Bacc(None, target_bir_lowering=False, debug=False)

    def din(name, shape, dt=F32):
        return nc.dram_tensor(name, shape, dt, kind="ExternalInput")

    x_in = din("x_fm", [2, 128, BS])
    h1_in = din("h1_fm", [2, 128, BS])
    c1_in = din("c1_fm", [2, 128, BS])
    h2_in = din("h2_fm", [2, 128, BS])
    c2_in = din("c2_fm", [2, 128, BS])
    w1ih_in = din("w1ihT", [2, 128, 4 * H])
    w1hh_in = din("w1hhT", [2, 128, 4 * H])
    w2ih_in = din("w2ihT", [2, 128, 4 * H])
    w2hh_in = din("w2hhT", [2, 128, 4 * H])
    fc1_in = din("fc1T", [2, 128, H])
    fc2_in = din("fc2T", [2, 128, H])
    fc3h_in = din("fc3Th", [2, 128, Q], BF16)
    fc3l_in = din("fc3Tl", [2, 128, Q], BF16)
    fcqw_in = din("fcqwT", [100, 128])
    fcql_in = din("fcqlT", [100, 128])
    b1_in = din("b1r", [128, 8])
    b2_in = din("b2r", [128, 8])
    fc1b_in = din("fc1br", [128, 2])
    fc2b_in = din("fc2br", [128, 2])
    b3rep_in = din("b3rep", [128, Q])
    fcqwb_in = din("fcqwb", [128, 1])
    fcqlb_in = din("fcqlb", [128, 1])

    idx_out = nc.dram_tensor("idx_out", [2, 128, 20], U32, kind="ExternalOutput")

    with tile.TileContext(nc) as tc:
        with (
            tc.tile_pool(name="wp", bufs=1) as wp,
            tc.tile_pool(name="st", bufs=1) as st,
            tc.tile_pool(name="wk", bufs=2) as wk,
            tc.tile_pool(name="ps", bufs=2, space="PSUM") as ps,
        ):
            # ---- weights / consts ----
            def wload(src, shape, tag, dt=F32):
                t = wp.tile(shape, dt, tag=tag, name=tag)
                if len(shape) == 3 and shape[1] == 2:
                    nc.sync.dma_start(t[:], src[:].rearrange("c p f -> p c f"))
                else:
                    nc.sync.dma_start(t[:], src[:])
                return t

            w1ih = wload(w1ih_in, [128, 2, 4 * H], "w1ih")
            w1hh = wload(w1hh_in, [128, 2, 4 * H], "w1hh")
            w2ih = wload(w2ih_in, [128, 2, 4 * H], "w2ih")
            w2hh = wload(w2hh_in, [128, 2, 4 * H], "w2hh")
            fc1 = wload(fc1_in, [128, 2, H], "fc1")
            fc2 = wload(fc2_in, [128, 2, H], "fc2")
            # fc3 weights: chunked DMA so group-0 matmuls start early
            fc3h = wp.tile([128, 2, Q], BF16, tag="fc3h", name="fc3h")
            fc3l = wp.tile([128, 2, Q], BF16, tag="fc3l", name="fc3l")
            for g in range(NG):
                sl = slice(GW * g, GW * (g + 1))
                nc.sync.dma_start(
                    fc3h[:, :, sl],
                    fc3h_in[:, :, sl].rearrange("c p f -> p c f"))
                nc.sync.dma_start(
                    fc3l[:, :, sl],
                    fc3l_in[:, :, sl].rearrange("c p f -> p c f"))
            fcqw = wload(fcqw_in, [100, 128], "fcqw")
            fcql = wload(fcql_in, [100, 128], "fcql")
            b1r = wload(b1_in, [128, 8], "b1r")
            b2r = wload(b2_in, [128, 8], "b2r")
            fc1b = wload(fc1b_in, [128, 2], "fc1b")
            fc2b = wload(fc2b_in, [128, 2], "fc2b")
            fcqwb = wload(fcqwb_in, [128, 1], "fcqwb")
            fcqlb = wload(fcqlb_in, [128, 1], "fcqlb")

            ident = wp.tile([128, 128], F32)
            make_identity(nc, ident[:])
            io_f = wp.tile([128, 100], F32)
            nc.gpsimd.iota(io_f[:], pattern=[[1, 100]], base=0,
                           channel_multiplier=0,
                           allow_small_or_imprecise_dtypes=True)
            io100 = wp.tile([128, 100], F32)
            nc.gpsimd.iota(io100[:], pattern=[[100, 100]], base=0,
                           channel_multiplier=0,
                           allow_small_or_imprecise_dtypes=True)
            io8 = wp.tile([128, 8], F32)
            nc.gpsimd.iota(io8[:], pattern=[[1, 8]], base=0,
                           channel_multiplier=0,
                           allow_small_or_imprecise_dtypes=True)


            # ---- PE warmup: real matmuls while DMAs land (HAM ignores
            # transpose-mode, so use matmul) ----
            for w in range(12):
                pw = ps.tile([128, 4, 512], F32, tag="big", name="big")
                nc.tensor.matmul(pw[:, 0, 0:128], ident[:], ident[:],
                                 start=True, stop=True)

            # ---- persistent state (feature-major [128, chunk, BS]) ----
            def sload(src, tag):
                t = st.tile([128, 2, BS], F32, tag=tag, name=tag)
                nc.sync.dma_start(t[:], src[:].rearrange("c p b -> p c b"))
                return t

            h1_t = sload(h1_in, "h1")
            c1_t = sload(c1_in, "c1")
            h2_t = sload(h2_in, "h2")
            c2_t = sload(c2_in, "c2")
            emb_t = st.tile([128, 2, BS], F32, tag="emb", name="emb")
            nc.sync.dma_start(emb_t[:], x_in[:].rearrange("c p b -> p c b"))
            outi = st.tile([128, 2, 20], U32, tag="outi", name="outi")
            nc.vector.memset(outi[:], 0)
            lq = st.tile([128, NG * 4, TW], F32, tag="lq", name="lq")

            def pbig():
                return ps.tile([128, 4, 512], F32, tag="big", name="big")

            def lstm_layer(inp, hT, cT, wih, whh, br):
                gpt = pbig()

                def gsl(g):
                    return gpt[:, g // 2, (g % 2) * 256:(g % 2) * 256 + 256]

                # hh first (indep of inp: scheduler hoists into prev step's
                # argmax tail); gates grouped by chunk for pointwise overlap
                for g in (0, 2, 4, 6, 1, 3, 5, 7):
                    sl = slice(128 * g, 128 * (g + 1))
                    nc.tensor.matmul(gsl(g), whh[:, 0, sl], hT[:, 0, :],
                                     start=True, stop=False)
                    nc.tensor.matmul(gsl(g), whh[:, 1, sl], hT[:, 1, :],
                                     start=False, stop=False)
                    nc.tensor.matmul(gsl(g), wih[:, 0, sl], inp[:, 0, :],
                                     start=False, stop=False)
                    nc.tensor.matmul(gsl(g), wih[:, 1, sl], inp[:, 1, :],
                                     start=False, stop=True)
                for ch in range(2):
                    si = wk.tile([128, 256], F32, tag="si", bufs=1)
                    sf = wk.tile([128, 256], F32, tag="sf", bufs=1)
                    tg = wk.tile([128, 256], F32, tag="tg", bufs=1)
                    so = wk.tile([128, 256], F32, tag="so", bufs=1)
                    nc.scalar.activation(si[:], gsl(0 + ch), AF.Sigmoid,
                                         bias=br[:, 0 + ch:1 + ch])
                    nc.scalar.activation(sf[:], gsl(2 + ch), AF.Sigmoid,
                                         bias=br[:, 2 + ch:3 + ch])
                    nc.scalar.activation(tg[:], gsl(4 + ch), AF.Tanh,
                                         bias=br[:, 4 + ch:5 + ch])
                    nc.scalar.activation(so[:], gsl(6 + ch), AF.Sigmoid,
                                         bias=br[:, 6 + ch:7 + ch])
                    t1 = wk.tile([128, 256], F32, tag="t1", bufs=1)
                    t2 = wk.tile([128, 256], F32, tag="t2", bufs=1)
                    nc.vector.tensor_mul(t1[:], sf[:], cT[:, ch, :])
                    nc.vector.tensor_mul(t2[:], si[:], tg[:])
                    nc.vector.tensor_add(cT[:, ch, :], t1[:], t2[:])
                    t3 = wk.tile([128, 256], F32, tag="t3", bufs=1)
                    nc.scalar.activation(t3[:], cT[:, ch, :], AF.Tanh)
                    nc.vector.tensor_mul(hT[:, ch, :], so[:], t3[:])

            for t in range(delta):
                lstm_layer(emb_t, h1_t, c1_t, w1ih, w1hh, b1r)
                lstm_layer(h1_t, h2_t, c2_t, w2ih, w2hh, b2r)

                # fc1, fc2 (feature-major out)
                y1 = st.tile([128, 2, BS], F32, tag="y1")
                fpt = pbig()
                for m in range(2):
                    fsl = fpt[:, m, 0:256]
                    sl = slice(128 * m, 128 * (m + 1))
                    nc.tensor.matmul(fsl, fc1[:, 0, sl], h2_t[:, 0, :],
                                     start=True, stop=False)
                    nc.tensor.matmul(fsl, fc1[:, 1, sl], h2_t[:, 1, :],
                                     start=False, stop=True)
                    nc.scalar.activation(y1[:, m, :], fsl, AF.Identity,
                                         bias=fc1b[:, m:m + 1])
                # fc2 evacuates straight into the bf16 hi/lo split
                y2h = st.tile([128, 2, BS], BF16, tag="y2h")
                y2l = st.tile([128, 2, BS], BF16, tag="y2l")
                fpt2 = pbig()
                for m in range(2):
                    fsl = fpt2[:, m, 0:256]
                    sl = slice(128 * m, 128 * (m + 1))
                    nc.tensor.matmul(fsl, fc2[:, 0, sl], y1[:, 0, :],
                                     start=True, stop=False)
                    nc.tensor.matmul(fsl, fc2[:, 1, sl], y1[:, 1, :],
                                     start=False, stop=True)
                    nc.scalar.activation(y2h[:, m, :], fsl, AF.Identity,
                                         bias=fc2b[:, m:m + 1])
                    nc.vector.scalar_tensor_tensor(
                        y2l[:, m, :], fsl, fc2b[:, m:m + 1], y2h[:, m, :],
                        op0=ALU.add, op1=ALU.subtract)

                ohwT = wk.tile([100, 256], F32, tag="ohwT", name="ohwT",
                               bufs=1)
                ohlT = wk.tile([100, 256], F32, tag="ohlT", name="ohlT",
                               bufs=1)
                for bc in range(2):
                    bsl = slice(128 * bc, 128 * (bc + 1))
                    for grp in range(NG):
                        # stream this group's fc3 bias (2000 cols, 1MB);
                        # DMA is idle during the step loop
                        b3g = wk.tile([128, 4, TW], F32, tag="b3g",
                                      name="b3g", bufs=2)
                        nc.sync.dma_start(
                            b3g[:],
                            b3rep_in[:, GW * grp:GW * (grp + 1)].rearrange(
                                "p (a b) -> p a b", a=4))
                        gp3 = pbig()
                        for tt in range(4):
                            n0 = (grp * 4 + tt) * TW
                            o = gp3[:, tt, 0:TW]
                            for k in range(2):
                                yhk = y2h[:, k, bsl]
                                ylk = y2l[:, k, bsl]
                                wh = fc3h[:, k, n0:n0 + TW]
                                wl = fc3l[:, k, n0:n0 + TW]
                                nc.tensor.matmul(o, yhk, wh, start=(k == 0),
                                                 stop=False)
                                nc.tensor.matmul(o, yhk, wl, start=False,
                                                 stop=False)
                                nc.tensor.matmul(o, ylk, wh, start=False,
                                                 stop=(k == 1))
                        # evac: lq = psum + bias
                        nc.vector.tensor_tensor(
                            lq[:, 4 * grp:4 * grp + 4, :],
                            gp3[:, 0:4, 0:TW],
                            b3g[:, 0:4, 0:TW],
                            op=ALU.add)

                    qsel = wk.tile([128, 4], F32, tag="qsel", name="qsel")
                    m8 = wk.tile([128, 8], F32, tag="m8", name="m8")
                    i8 = wk.tile([128, 8], U32, tag="i8", name="i8")
                    lqf = lq[:].rearrange("p a b -> p (a b)")
                    nc.vector.max(m8[:], lqf)
                    nc.vector.max_index(i8[:], m8[:], lqf)
                    nc.vector.tensor_copy(qsel[:], i8[:, 0:4])

                    if t == 0:
                        nc.vector.tensor_copy(outi[:, bc, 0:4], qsel[:, 0:4])
                    else:
                        nc.vector.tensor_copy(outi[:, bc, 4 + t - 1:5 + t - 1],
                                              qsel[:, 0:1])
                    if t == delta - 1:
                        continue
                    # embedding one-hots for beam 0
                    qf = wk.tile([128, 1], F32, tag="qf", name="qf")
                    nc.vector.tensor_copy(qf[:], qsel[:, 0:1])
                    m_ge = wk.tile([128, 100], F32, tag="mge", bufs=1)
                    nc.vector.tensor_scalar(m_ge[:], io100[:], qf[:], None,
                                            op0=ALU.is_le)
                    qm = wk.tile([128, 1], F32, tag="qm", name="qm")
                    nc.vector.tensor_scalar(qm[:], qf[:], -100.0, None,
                                            op0=ALU.add)
                    m_lt = wk.tile([128, 100], F32, tag="mlt", bufs=1)
                    nc.vector.tensor_scalar(m_lt[:], io100[:], qm[:], None,
                                            op0=ALU.is_gt)
                    ohw = wk.tile([128, 100], F32, tag="ohw", bufs=1)
                    nc.vector.tensor_mul(ohw[:], m_ge[:], m_lt[:])
                    tm = wk.tile([128, 100], F32, tag="tm", bufs=1)
                    nc.vector.tensor_mul(tm[:], ohw[:], io_f[:])
                    fwf = wk.tile([128, 1], F32, tag="fwf", name="fwf")
                    nc.vector.tensor_reduce(fwf[:], tm[:],
                                            axis=mybir.AxisListType.X,
                                            op=ALU.add)
                    flf = wk.tile([128, 1], F32, tag="flf", name="flf")
                    nc.vector.tensor_scalar(flf[:], fwf[:], -100.0, qf[:],
                                            op0=ALU.mult, op1=ALU.add)
                    ohl = wk.tile([128, 100], F32, tag="ohl", bufs=1)
                    nc.vector.tensor_scalar(ohl[:], io_f[:], flf[:], None,
                                            op0=ALU.is_equal)
                    ptr = pbig()
                    pw = ptr[0:100, 0, 0:128]
                    nc.tensor.transpose(pw, ohw[:], ident[:])
                    nc.vector.tensor_copy(ohwT[:, bsl], pw)
                    pl = ptr[0:100, 1, 0:128]
                    nc.tensor.transpose(pl, ohl[:], ident[:])
                    nc.vector.tensor_copy(ohlT[:, bsl], pl)

                if t == delta - 1:
                    continue
                # embedding gather matmuls + bias
                pet = pbig()
                pe0 = pet[:, 0, 0:BS]
                pe1 = pet[:, 1, 0:BS]
                nc.tensor.matmul(pe0, fcqw[:], ohwT[:], start=True, stop=True)
                nc.tensor.matmul(pe1, fcql[:], ohlT[:], start=True, stop=True)
                nc.scalar.activation(emb_t[:, 0, :], pe0, AF.Identity,
                                     bias=fcqwb[:])
                nc.scalar.activation(emb_t[:, 1, :], pe1, AF.Identity,
                                     bias=fcqlb[:])

            for bc in range(2):
                nc.sync.dma_start(idx_out[bc], outi[:, bc, :])
    nc.finalize()
    return nc


def _prep_shared(inputs):
    f32 = np.float32
    bf = ml_dtypes.bfloat16

    def fm(w):  # [out,in] -> lhsT layout [2,128,out]
        wt = np.ascontiguousarray(w.T.astype(f32))        # [in, out]
        return wt.reshape(2, 128, wt.shape[1])

    fc3T = np.ascontiguousarray(inputs["fc3_W"].T.astype(f32))  # [256, 10000]
    fc3h = fc3T.astype(bf)
    fc3l = (fc3T - fc3h.astype(f32)).astype(bf)

    shared = {
        "w1ihT": fm(inputs["lstm1_Wih"]),
        "w1hhT": fm(inputs["lstm1_Whh"]),
        "w2ihT": fm(inputs["lstm2_Wih"]),
        "w2hhT": fm(inputs["lstm2_Whh"]),
        "fc1T": fm(inputs["fc1_W"]),
        "fc2T": fm(inputs["fc2_W"]),
        "fc3Th": fc3h.reshape(2, 128, Q),
        "fc3Tl": fc3l.reshape(2, 128, Q),
        "fcqwT": np.ascontiguousarray(inputs["fcqw_W"].T.astype(f32))[:, :],
        "fcqlT": np.ascontiguousarray(inputs["fcql_W"].T.astype(f32))[:, :],
        "b1r": inputs["lstm1_b"].astype(f32).reshape(8, 128).T.copy(),
        "b2r": inputs["lstm2_b"].astype(f32).reshape(8, 128).T.copy(),
        "fc1br": inputs["fc1_b"].astype(f32).reshape(2, 128).T.copy(),
        "fc2br": inputs["fc2_b"].astype(f32).reshape(2, 128).T.copy(),
        "b3rep": np.ascontiguousarray(
            np.broadcast_to(inputs["fc3_b"].astype(f32), (128, Q))),
        "fcqwb": inputs["fcqw_b"].astype(f32).reshape(128, 1),
        "fcqlb": inputs["fcql_b"].astype(f32).reshape(128, 1),
    }
    return shared


def _per_core(inputs, c):
    f32 = np.float32
    sl = slice(c * BS, (c + 1) * BS)

    def fmT(a):  # [BS, 256] -> [2, 128, BS]
        return np.ascontiguousarray(a.T.astype(f32)).reshape(2, 128, BS)

    return {
        "x_fm": fmT(inputs["x"][sl, 0, :]),
        "h1_fm": fmT(inputs["h1"][0, sl]),
        "c1_fm": fmT(inputs["c1"][0, sl]),
        "h2_fm": fmT(inputs["h2"][0, sl]),
        "c2_fm": fmT(inputs["c2"][0, sl]),
    }


def kernel(**inputs):
    key = "nc"
    if key not in _CACHE:
        _CACHE[key] = _build_nc()
    nc = _CACHE[key]

    shared = _prep_shared(inputs)
    in_maps = []
    for c in range(NCORES):
        m = dict(shared)
        m.update(_per_core(inputs, c))
        in_maps.append(m)

    from concourse.bass_utils import run_bass_kernel_spmd
    res = run_bass_kernel_spmd(nc, in_maps, list(range(NCORES)))
    return assemble(res.results)


def assemble(results):
    traj = np.zeros((B, DELTA, K4, 2), np.float32)
    for c, r in enumerate(results):
        idx = r["idx_out"].reshape(2, 128, 20).astype(np.int64)
        for bc in range(2):
            rows = slice(c * BS + bc * 128, c * BS + (bc + 1) * 128)
            top4 = idx[bc, :, 0:4]
            traj[rows, 0, :, 0] = (top4 % QL).astype(np.float32)
            traj[rows, 0, :, 1] = (top4 // QL).astype(np.float32)
            greedy = idx[bc, :, 4:4 + DELTA - 1]
            traj[rows, 1:, 0, 0] = (greedy % QL).astype(np.float32)
            traj[rows, 1:, 0, 1] = (greedy // QL).astype(np.float32)
    return traj


# revision 34
# speedup vs baseline: 1.4489x; 1.2287x over previous
"""Trainium2 Bass kernel for nn_Decoder (2-layer LSTM + 3 FC + top-k decode).

Data parallel over batch (2048 -> 8 cores x 256). Activations feature-major
[feat, batch]. LSTM/fc1/fc2 fp32 on PE (exactness for the recurrence);
fc3 (256->10000) as 3-term bf16 split (err ~1e-6 rel, enough for argmax).
Argmax: fused DVE tensor_tensor_reduce (bias add + per-group max) evacuates
psum -> lq, winner group located from the 5 group maxes, gpsimd
indirect_copy gathers the winning 2000-wide window, short max_index gives
the index. Host assembles the [B,16,4,2] trajectory from indices.
"""
import numpy as np
import ml_dtypes

B, D, H = 2048, 256, 256
K4, QW, QL, DELTA = 4, 100, 100, 16
Q = QW * QL
NCORES = 8
BS = B // NCORES          # 256 rows per core
TW = 512                  # fc3 N-tile width (full psum bank)
NG = 5                    # fc3 groups of 4 tiles
GW = 4 * TW               # group width (2048)
QP = NG * GW              # padded Q = 10240 (pad logits get bias -1e30)

_CACHE = {}


def _build_nc(delta=DELTA):
    import concourse.mybir as mybir
    import concourse.tile as tile
    import concourse.bacc as bacc
    from concourse.masks import make_identity

    F32 = mybir.dt.float32
    BF16 = mybir.dt.bfloat16
    U32 = mybir.dt.uint32
    U16 = mybir.dt.uint16
    AF = mybir.ActivationFunctionType
    ALU = mybir.AluOpType

    nc = bacc.Bacc(None, target_bir_lowering=False, debug=False)

    def din(name, shape, dt=F32):
        return nc.dram_tensor(name, shape, dt, kind="ExternalInput")

    x_in = din("x_fm", [2, 128, BS])
    h1_in = din("h1_fm", [2, 128, BS])
    c1_in = din("c1_fm", [2, 128, BS])
    h2_in = din("h2_fm", [2, 128, BS])
    c2_in = din("c2_fm", [2, 128, BS])
    w1ih_in = din("w1ihT", [2, 128, 4 * H])
    w1hh_in = din("w1hhT", [2, 128, 4 * H])
    w2ih_in = din("w2ihT", [2, 128, 4 * H])
    w2hh_in = din("w2hhT", [2, 128, 4 * H])
    fc1_in = din("fc1T", [2, 128, H])
    fc2_in = din("fc2T", [2, 128, H])
    fc3h_in = din("fc3Th", [2, 128, QP], BF16)
    fc3l_in = din("fc3Tl", [2, 128, QP], BF16)
    fcqw_in = din("fcqwT", [100, 128])
    fcql_in = din("fcqlT", [100, 128])
    b1_in = din("b1r", [128, 8])
    b2_in = din("b2r", [128, 8])
    fc1b_in = din("fc1br", [128, 2])
    fc2b_in = din("fc2br", [128, 2])
    b3rep_in = din("b3rep", [128, QP])
    fcqwb_in = din("fcqwb", [128, 1])
    fcqlb_in = din("fcqlb", [128, 1])

    idx_out = nc.dram_tensor("idx_out", [2, 128, 20], U32, kind="ExternalOutput")

    with tile.TileContext(nc) as tc:
        with (
            tc.tile_pool(name="wp", bufs=1) as wp,
            tc.tile_pool(name="st", bufs=1) as st,
            tc.tile_pool(name="wk", bufs=2) as wk,
            tc.tile_pool(name="ps", bufs=2, space="PSUM") as ps,
        ):
            # ---- weights / consts ----
            def wload(src, shape, tag, dt=F32):
                t = wp.tile(shape, dt, tag=tag, name=tag)
                if len(shape) == 3 and shape[1] == 2:
                    nc.sync.dma_start(t[:], src[:].rearrange("c p f -> p c f"))
                else:
                    nc.sync.dma_start(t[:], src[:])
                return t

            w1ih = wload(w1ih_in, [128, 2, 4 * H], "w1ih")
            w1hh = wload(w1hh_in, [128, 2, 4 * H], "w1hh")
            w2ih = wload(w2ih_in, [128, 2, 4 * H], "w2ih")
            w2hh = wload(w2hh_in, [128, 2, 4 * H], "w2hh")
            fc1 = wload(fc1_in, [128, 2, H], "fc1")
            fc2 = wload(fc2_in, [128, 2, H], "fc2")
            fc3h = wp.tile([128, 2, QP], BF16, tag="fc3h", name="fc3h")
            fc3l = wp.tile([128, 2, QP], BF16, tag="fc3l", name="fc3l")
            fcqw = wload(fcqw_in, [100, 128], "fcqw")
            fcql = wload(fcql_in, [100, 128], "fcql")
            b1r = wload(b1_in, [128, 8], "b1r")
            b2r = wload(b2_in, [128, 8], "b2r")
            fc1b = wload(fc1b_in, [128, 2], "fc1b")
            fc2b = wload(fc2b_in, [128, 2], "fc2b")
            b3rep = wp.tile([128, NG * 4, TW], F32, tag="b3rep", name="b3rep")
            nc.sync.dma_start(
                b3rep[:], b3rep_in[:].rearrange("p (a b) -> p a b", a=NG * 4))
            fcqwb = wload(fcqwb_in, [128, 1], "fcqwb")
            fcqlb = wload(fcqlb_in, [128, 1], "fcqlb")

            ident = wp.tile([128, 128], F32)
            make_identity(nc, ident[:])
            io_f = wp.tile([128, 100], F32)
            nc.gpsimd.iota(io_f[:], pattern=[[1, 100]], base=0,
                           channel_multiplier=0,
                           allow_small_or_imprecise_dtypes=True)
            io100 = wp.tile([128, 100], F32)
            nc.gpsimd.iota(io100[:], pattern=[[100, 100]], base=0,
                           channel_multiplier=0,
                           allow_small_or_imprecise_dtypes=True)
            io40 = wp.tile([128, 40], F32)
            nc.gpsimd.iota(io40[:], pattern=[[1, 40]], base=0,
                           channel_multiplier=0,
                           allow_small_or_imprecise_dtypes=True)
            # per-candidate global column offsets: (slot//8)*GW
            gofs = wp.tile([128, 5, 8], F32)
            nc.gpsimd.iota(gofs[:], pattern=[[GW, 5], [0, 8]], base=0,
                           channel_multiplier=0,
                           allow_small_or_imprecise_dtypes=True)


            # ---- PE warmup: real matmuls while DMAs land (HAM ignores
            # transpose-mode, so use matmul) ----
            for w in range(12):
                pw = ps.tile([128, 4, 512], F32, tag="big", name="big")
                nc.tensor.matmul(pw[:, 0, 0:128], ident[:], ident[:],
                                 start=True, stop=True)

            # ---- persistent state (feature-major [128, chunk, BS]) ----
            def sload(src, tag):
                t = st.tile([128, 2, BS], F32, tag=tag, name=tag)
                nc.sync.dma_start(t[:], src[:].rearrange("c p b -> p c b"))
                return t

            h1_t = sload(h1_in, "h1")
            c1_t = sload(c1_in, "c1")
            h2_t = sload(h2_in, "h2")
            c2_t = sload(c2_in, "c2")
            emb_t = st.tile([128, 2, BS], F32, tag="emb", name="emb")
            nc.sync.dma_start(emb_t[:], x_in[:].rearrange("c p b -> p c b"))
            # fc3 weights last (chunked): states/LSTM weights land first so
            # step 0 starts immediately; group-0 fc3 tiles arrive in time
            for g in range(NG):
                sl = slice(GW * g, GW * (g + 1))
                nc.sync.dma_start(
                    fc3h[:, :, sl],
                    fc3h_in[:, :, sl].rearrange("c p f -> p c f"))
                nc.sync.dma_start(
                    fc3l[:, :, sl],
                    fc3l_in[:, :, sl].rearrange("c p f -> p c f"))
            outi = st.tile([128, 2, 20], U32, tag="outi", name="outi")
            nc.vector.memset(outi[:], 0)

            def pbig():
                return ps.tile([128, 4, 512], F32, tag="big", name="big")

            def lstm_layer(inp, hT, cT, wih, whh, br):
                gpt = pbig()

                def gsl(g):
                    return gpt[:, g // 2, (g % 2) * 256:(g % 2) * 256 + 256]

                # hh first (indep of inp: scheduler hoists into prev step's
                # argmax tail); gates grouped by chunk for pointwise overlap
                for g in (0, 2, 4, 6, 1, 3, 5, 7):
                    sl = slice(128 * g, 128 * (g + 1))
                    nc.tensor.matmul(gsl(g), whh[:, 0, sl], hT[:, 0, :],
                                     start=True, stop=False)
                    nc.tensor.matmul(gsl(g), whh[:, 1, sl], hT[:, 1, :],
                                     start=False, stop=False)
                    nc.tensor.matmul(gsl(g), wih[:, 0, sl], inp[:, 0, :],
                                     start=False, stop=False)
                    nc.tensor.matmul(gsl(g), wih[:, 1, sl], inp[:, 1, :],
                                     start=False, stop=True)
                for ch in range(2):
                    si = wk.tile([128, 256], F32, tag="si", bufs=1)
                    sf = wk.tile([128, 256], F32, tag="sf", bufs=1)
                    tg = wk.tile([128, 256], F32, tag="tg", bufs=1)
                    so = wk.tile([128, 256], F32, tag="so", bufs=1)
                    nc.scalar.activation(si[:], gsl(0 + ch), AF.Sigmoid,
                                         bias=br[:, 0 + ch:1 + ch])
                    nc.scalar.activation(sf[:], gsl(2 + ch), AF.Sigmoid,
                                         bias=br[:, 2 + ch:3 + ch])
                    nc.scalar.activation(tg[:], gsl(4 + ch), AF.Tanh,
                                         bias=br[:, 4 + ch:5 + ch])
                    nc.scalar.activation(so[:], gsl(6 + ch), AF.Sigmoid,
                                         bias=br[:, 6 + ch:7 + ch])
                    t1 = wk.tile([128, 256], F32, tag="t1", bufs=1)
                    t2 = wk.tile([128, 256], F32, tag="t2", bufs=1)
                    nc.vector.tensor_mul(t1[:], sf[:], cT[:, ch, :])
                    nc.vector.tensor_mul(t2[:], si[:], tg[:])
                    nc.vector.tensor_add(cT[:, ch, :], t1[:], t2[:])
                    t3 = wk.tile([128, 256], F32, tag="t3", bufs=1)
                    nc.scalar.activation(t3[:], cT[:, ch, :], AF.Tanh)
                    nc.vector.tensor_mul(hT[:, ch, :], so[:], t3[:])

            for t in range(delta):
                lstm_layer(emb_t, h1_t, c1_t, w1ih, w1hh, b1r)
                lstm_layer(h1_t, h2_t, c2_t, w2ih, w2hh, b2r)

                # fc1, fc2 (feature-major out)
                y1 = st.tile([128, 2, BS], F32, tag="y1")
                fpt = pbig()
                for m in range(2):
                    fsl = fpt[:, m, 0:256]
                    sl = slice(128 * m, 128 * (m + 1))
                    nc.tensor.matmul(fsl, fc1[:, 0, sl], h2_t[:, 0, :],
                                     start=True, stop=False)
                    nc.tensor.matmul(fsl, fc1[:, 1, sl], h2_t[:, 1, :],
                                     start=False, stop=True)
                    nc.scalar.activation(y1[:, m, :], fsl, AF.Identity,
                                         bias=fc1b[:, m:m + 1])
                # fc2 evacuates straight into the bf16 hi/lo split
                y2h = st.tile([128, 2, BS], BF16, tag="y2h")
                y2l = st.tile([128, 2, BS], BF16, tag="y2l")
                fpt2 = pbig()
                for m in range(2):
                    fsl = fpt2[:, m, 0:256]
                    sl = slice(128 * m, 128 * (m + 1))
                    nc.tensor.matmul(fsl, fc2[:, 0, sl], y1[:, 0, :],
                                     start=True, stop=False)
                    nc.tensor.matmul(fsl, fc2[:, 1, sl], y1[:, 1, :],
                                     start=False, stop=True)
                    nc.scalar.activation(y2h[:, m, :], fsl, AF.Identity,
                                         bias=fc2b[:, m:m + 1])
                    nc.vector.scalar_tensor_tensor(
                        y2l[:, m, :], fsl, fc2b[:, m:m + 1], y2h[:, m, :],
                        op0=ALU.add, op1=ALU.subtract)

                ohwT = wk.tile([100, 256], F32, tag="ohwT", name="ohwT",
                               bufs=1)
                ohlT = wk.tile([100, 256], F32, tag="ohlT", name="ohlT",
                               bufs=1)
                cands = []
                for bc in range(2):
                    cands.append((
                        wk.tile([128, 5, 8], F32, tag=f"candv{bc}",
                                name=f"candv{bc}", bufs=1),
                        wk.tile([128, 5, 8], U32, tag=f"candi{bc}",
                                name=f"candi{bc}", bufs=1)))
                for grp in range(NG):
                    for bc in range(2):
                        bsl = slice(128 * bc, 128 * (bc + 1))
                        candv, candi = cands[bc]
                        gp3 = pbig()
                        for tt in range(4):
                            n0 = (grp * 4 + tt) * TW
                            o = gp3[:, tt, 0:TW]
                            for k in range(2):
                                yhk = y2h[:, k, bsl]
                                ylk = y2l[:, k, bsl]
                                wh = fc3h[:, k, n0:n0 + TW]
                                wl = fc3l[:, k, n0:n0 + TW]
                                nc.tensor.matmul(o, yhk, wh, start=(k == 0),
                                                 stop=False)
                                nc.tensor.matmul(o, yhk, wl, start=False,
                                                 stop=False)
                                nc.tensor.matmul(o, ylk, wh, start=False,
                                                 stop=(k == 1))
                        # evac this group (+bias) to a small ring, then
                        # per-group top-8 values+indices (keeps lq tiny and
                        # never blocks the pipeline on a full-width scan)
                        lqg = wk.tile([128, 4, TW], F32, tag="lqg",
                                      name="lqg", bufs=2)
                        nc.vector.tensor_tensor(
                            lqg[:], gp3[:, 0:4, 0:TW],
                            b3rep[:, 4 * grp:4 * grp + 4, :], op=ALU.add)
                        gflat = lqg[:].rearrange("p a b -> p (a b)")
                        nc.vector.max(candv[:, grp, :], gflat)
                        nc.vector.max_index(candi[:, grp, :], candv[:, grp, :],
                                            gflat)

                for bc in range(2):
                    bsl = slice(128 * bc, 128 * (bc + 1))
                    candv, candi = cands[bc]
                    # merge the 40 candidates
                    candif = wk.tile([128, 5, 8], F32, tag="candif",
                                     name="candif")
                    nc.vector.tensor_copy(candif[:], candi[:])
                    candg = wk.tile([128, 5, 8], F32, tag="candg",
                                    name="candg")
                    nc.vector.tensor_tensor(candg[:], candif[:], gofs[:],
                                            op=ALU.add)
                    cvf = candv[:].rearrange("p a b -> p (a b)")
                    cgf = candg[:].rearrange("p a b -> p (a b)")
                    qsel = wk.tile([128, 4], F32, tag="qsel", name="qsel")
                    if t == 0:
                        m8 = wk.tile([128, 8], F32, tag="m8", name="m8")
                        i8 = wk.tile([128, 8], U32, tag="i8", name="i8")
                        nc.vector.max(m8[:], cvf)
                        nc.vector.max_index(i8[:], m8[:], cvf)
                        i8f = wk.tile([128, 8], F32, tag="i8f", name="i8f")
                        nc.vector.tensor_copy(i8f[:], i8[:])
                        ohp = wk.tile([128, 40], F32, tag="ohp", name="ohp")
                        tmq = wk.tile([128, 40], F32, tag="tmq", name="tmq")
                        for kk in range(4):
                            nc.vector.tensor_scalar(
                                ohp[:], io40[:], i8f[:, kk:kk + 1], None,
                                op0=ALU.is_equal)
                            nc.vector.tensor_mul(tmq[:], ohp[:], cgf)
                            nc.vector.tensor_reduce(
                                qsel[:, kk:kk + 1], tmq[:],
                                axis=mybir.AxisListType.X, op=ALU.add)
                    else:
                        gm = wk.tile([128, 1], F32, tag="gm", name="gm")
                        nc.vector.tensor_reduce(gm[:], cvf,
                                                axis=mybir.AxisListType.X,
                                                op=ALU.max)
                        iseq = wk.tile([128, 40], F32, tag="iseq",
                                       name="iseq")
                        nc.vector.tensor_scalar(iseq[:], cvf, gm[:], None,
                                                op0=ALU.is_equal)
                        qc = wk.tile([128, 40], F32, tag="qc", name="qc")
                        nc.vector.scalar_tensor_tensor(
                            qc[:], iseq[:], -32768.0, cgf,
                            op0=ALU.mult, op1=ALU.add)
                        qmn = wk.tile([128, 1], F32, tag="qmn", name="qmn")
                        nc.vector.tensor_reduce(qmn[:], qc[:],
                                                axis=mybir.AxisListType.X,
                                                op=ALU.min)
                        nc.vector.tensor_scalar(qsel[:, 0:1], qmn[:], 32768.0,
                                                None, op0=ALU.add)

                    if t == 0:
                        nc.vector.tensor_copy(outi[:, bc, 0:4], qsel[:, 0:4])
                    else:
                        nc.vector.tensor_copy(outi[:, bc, 4 + t - 1:5 + t - 1],
                                              qsel[:, 0:1])
                    if t == delta - 1:
                        continue
                    # embedding one-hots for beam 0
                    qf = wk.tile([128, 1], F32, tag="qf", name="qf")
                    nc.vector.tensor_copy(qf[:], qsel[:, 0:1])
                    m_ge = wk.tile([128, 100], F32, tag="mge", bufs=1)
                    nc.vector.tensor_scalar(m_ge[:], io100[:], qf[:], None,
                                            op0=ALU.is_le)
                    qm = wk.tile([128, 1], F32, tag="qm", name="qm")
                    nc.vector.tensor_scalar(qm[:], qf[:], -100.0, None,
                                            op0=ALU.add)
                    m_lt = wk.tile([128, 100], F32, tag="mlt", bufs=1)
                    nc.vector.tensor_scalar(m_lt[:], io100[:], qm[:], None,
                                            op0=ALU.is_gt)
                    ohw = wk.tile([128, 100], F32, tag="ohw", bufs=1)
                    nc.vector.tensor_mul(ohw[:], m_ge[:], m_lt[:])
                    tm = wk.tile([128, 100], F32, tag="tm", bufs=1)
                    nc.vector.tensor_mul(tm[:], ohw[:], io_f[:])
                    fwf = wk.tile([128, 1], F32, tag="fwf", name="fwf")
                    nc.vector.tensor_reduce(fwf[:], tm[:],
                                            axis=mybir.AxisListType.X,
                                            op=ALU.add)
                    flf = wk.tile([128, 1], F32, tag="flf", name="flf")
                    nc.vector.tensor_scalar(flf[:], fwf[:], -100.0, qf[:],
                                            op0=ALU.mult, op1=ALU.add)
                    ohl = wk.tile([128, 100], F32, tag="ohl", bufs=1)
                    nc.vector.tensor_scalar(ohl[:], io_f[:], flf[:], None,
                                            op0=ALU.is_equal)
                    ptr = pbig()
                    pw = ptr[0:100, 0, 0:128]
                    nc.tensor.transpose(pw, ohw[:], ident[:])
                    nc.vector.tensor_copy(ohwT[:, bsl], pw)
                    pl = ptr[0:100, 1, 0:128]
                    nc.tensor.transpose(pl, ohl[:], ident[:])
                    nc.vector.tensor_copy(ohlT[:, bsl], pl)

                if t == delta - 1:
                    continue
                # embedding gather matmuls + bias
                pet = pbig()
                pe0 = pet[:, 0, 0:BS]
                pe1 = pet[:, 1, 0:BS]
                nc.tensor.matmul(pe0, fcqw[:], ohwT[:], start=True, stop=True)
                nc.tensor.matmul(pe1, fcql[:], ohlT[:], start=True, stop=True)
                nc.scalar.activation(emb_t[:, 0, :], pe0, AF.Identity,
                                     bias=fcqwb[:])
                nc.scalar.activation(emb_t[:, 1, :], pe1, AF.Identity,
                                     bias=fcqlb[:])

            for bc in range(2):
                nc.sync.dma_start(idx_out[bc], outi[:, bc, :])
    nc.finalize()
    return nc


def _prep_shared(inputs):
    f32 = np.float32
    bf = ml_dtypes.bfloat16

    def fm(w):  # [out,in] -> lhsT layout [2,128,out]
        wt = np.ascontiguousarray(w.T.astype(f32))        # [in, out]
        return wt.reshape(2, 128, wt.shape[1])

    fc3T = np.zeros((256, QP), f32)
    fc3T[:, :Q] = inputs["fc3_W"].T.astype(f32)
    fc3h = fc3T.astype(bf)
    fc3l = (fc3T - fc3h.astype(f32)).astype(bf)
    b3p = np.full((QP,), -1e30, f32)
    b3p[:Q] = inputs["fc3_b"].astype(f32)

    shared = {
        "w1ihT": fm(inputs["lstm1_Wih"]),
        "w1hhT": fm(inputs["lstm1_Whh"]),
        "w2ihT": fm(inputs["lstm2_Wih"]),
        "w2hhT": fm(inputs["lstm2_Whh"]),
        "fc1T": fm(inputs["fc1_W"]),
        "fc2T": fm(inputs["fc2_W"]),
        "fc3Th": fc3h.reshape(2, 128, QP),
        "fc3Tl": fc3l.reshape(2, 128, QP),
        "fcqwT": np.ascontiguousarray(inputs["fcqw_W"].T.astype(f32))[:, :],
        "fcqlT": np.ascontiguousarray(inputs["fcql_W"].T.astype(f32))[:, :],
        "b1r": inputs["lstm1_b"].astype(f32).reshape(8, 128).T.copy(),
        "b2r": inputs["lstm2_b"].astype(f32).reshape(8, 128).T.copy(),
        "fc1br": inputs["fc1_b"].astype(f32).reshape(2, 128).T.copy(),
        "fc2br": inputs["fc2_b"].astype(f32).reshape(2, 128).T.copy(),
        "b3rep": np.ascontiguousarray(np.broadcast_to(b3p, (128, QP))),
        "fcqwb": inputs["fcqw_b"].astype(f32).reshape(128, 1),
        "fcqlb": inputs["fcql_b"].astype(f32).reshape(128, 1),
    }
    return shared


def _per_core(inputs, c):
    f32 = np.float32
    sl = slice(c * BS, (c + 1) * BS)

    def fmT(a):  # [BS, 256] -> [2, 128, BS]
        return np.ascontiguousarray(a.T.astype(f32)).reshape(2, 128, BS)

    return {
        "x_fm": fmT(inputs["x"][sl, 0, :]),
        "h1_fm": fmT(inputs["h1"][0, sl]),
        "c1_fm": fmT(inputs["c1"][0, sl]),
        "h2_fm": fmT(inputs["h2"][0, sl]),
        "c2_fm": fmT(inputs["c2"][0, sl]),
    }


def kernel(**inputs):
    key = "nc"
    if key not in _CACHE:
        _CACHE[key] = _build_nc()
    nc = _CACHE[key]

    shared = _prep_shared(inputs)
    in_maps = []
    for c in range(NCORES):
        m = dict(shared)
        m.update(_per_core(inputs, c))
        in_maps.append(m)

    from concourse.bass_utils import run_bass_kernel_spmd
    res = run_bass_kernel_spmd(nc, in_maps, list(range(NCORES)))
    return assemble(res.results)


def assemble(results):
    traj = np.zeros((B, DELTA, K4, 2), np.float32)
    for c, r in enumerate(results):
        idx = r["idx_out"].reshape(2, 128, 20).astype(np.int64)
        for bc in range(2):
            rows = slice(c * BS + bc * 128, c * BS + (bc + 1) * 128)
            top4 = idx[bc, :, 0:4]
            traj[rows, 0, :, 0] = (top4 % QL).astype(np.float32)
            traj[rows, 0, :, 1] = (top4 // QL).astype(np.float32)
            greedy = idx[bc, :, 4:4 + DELTA - 1]
            traj[rows, 1:, 0, 0] = (greedy % QL).astype(np.float32)
            traj[rows, 1:, 0, 1] = (greedy // QL).astype(np.float32)
    return traj
